# revision 19
# baseline (speedup 1.0000x reference)
"""PVT-style spatial-reduction attention on 8 Trainium2 NeuronCores.

Sharding: data-parallel over batch (B=8 -> one batch element per core).
Each core runs the full attention for its batch element; weights are
replicated. No collectives needed.

v3 schedule (v2 was ACT-exp-limited at ~91% PE busy; this version attacks
both PE cycles and the elementwise wall):
  - scores matmuls have K=hd=64, so the two heads of a pair run
    CONCURRENTLY in the PE array via row tiling (tile_position (0,0) and
    (64,0) auto-derived from base partitions) -> ~2x on the score phase.
    Head 4 pairs its own kv-chunks against row-duplicated qT[2]/kT[2]
    (the duplicates are written by col-tiled double matmuls, ~free).
  - exp: most steps on ACT ([128,1024] ACTIVATE per head); a fraction on
    DVE as a single tensor_scalar Schraudolph exp straight into f16 bits
    (i16 = round(184.665*score + 10868.74); +-3% per weight, cancels in
    softmax; validated 1e-2 rel err at 100% usage, we use ~30%).
  - softmax normalization: denominators ride the av matmul (ones column),
    one reciprocal [1,1024] + gpsimd partition_broadcast + ONE DVE
    multiply reading the av psum directly (no staging copy).
  - output projection bias is folded in via a ones row in attnT[2] and a
    bp row in the Wp blob; qproj psum->sbuf copies ride the (ramp-idle)
    ACT engine.
  - PSUM: pool "s" = scores/prep ring (2x[128,1024]f32 = 4 banks),
    pool "a" = conv/av/proj ring (2x[128,1024]f32 = 4 banks).
"""

import os
import sys
from collections import deque

import numpy as np

for _p in ("/opt/trn_rl_repo", "/root/.axon_site/_ro/trn_rl_repo"):
    if os.path.isdir(_p) and _p not in sys.path:
        sys.path.append(_p)

import concourse.bacc as bacc
import concourse.bass as bass
import concourse.mybir as mybir
import concourse.tile as tile
from concourse.bass_utils import run_bass_kernel_spmd
from concourse.masks import make_identity

F16 = mybir.dt.float16
F32 = mybir.dt.float32
I16 = mybir.dt.int16
I32 = mybir.dt.int32

N = 4096          # q tokens (H*W = 64*64)
C = 320           # model dim
NH = 5            # heads
HD = 64           # head dim
NP = 1024         # kv tokens ((H/2)*(W/2))
QB = 512
LN_EPS = 1e-3
SCALE = HD ** -0.5
# NOTE: bias must be 0: raw scores reach +-65, and a negative Schraudolph
# int16 bitcasts to a LARGE negative f16 (not a tiny one), nuking the
# softmax. With bias 0 the i16 range is [3187, 27305] - safe both ends.
EXP_BIAS = 0.0

# Schraudolph f16 exp: i16 = round(A*(SCALE*x+EXP_BIAS) + (15360-59.3))
SCH_A = 1477.3195
SCH_MUL = SCH_A * SCALE
SCH_ADD = 15360.0 - 59.3 + SCH_A * EXP_BIAS

# contraction chunks over C=320: three 128-row tiles; the last one holds
# c 192:320 and uses rows 64:128 (its top 64 rows overlap chunk 1).
CCHUNKS = [(0, 0, 128), (128, 0, 128), (192, 64, 128)]  # (c_start, row0, rows)
# output chunks over C=320
OCHUNKS = [(0, 128), (128, 128), (256, 64)]

# which attention steps compute exp on DVE (Schraudolph) instead of ACT.
# NOTE: v3 measured that routing exp through the busy DVE FIFO adds ~2-4us
# latency on those steps; the resulting periodic PE stalls tripped the HAM
# clock gate into K=4/8 oscillation (88us cold windows). Disabled until the
# DVE queue is restructured.
DVE_EXP_MOD = 1 << 30
DVE_EXP_OFF = 1


def build_bass(dbg=False):
    nc = bacc.Bacc("TRN2", target_bir_lowering=False, debug=False, num_devices=8)

    xdt_d = nc.declare_dram_parameter("xdt", [C, N], F16, isOutput=False)
    wba_d = nc.declare_dram_parameter("wba", [128, 4800], F16, isOutput=False)
    wbb_d = nc.declare_dram_parameter("wbb", [128, 2880], F16, isOutput=False)
    wb32_d = nc.declare_dram_parameter("wb32", [128, 963], F32, isOutput=False)
    out_d = nc.declare_dram_parameter("out", [N, C], F16, isOutput=True)
    dbg_d = {}
    if dbg:
        for nm, shp in [("dbg_kt0", [128, NP]), ("dbg_kt2", [128, NP]),
                        ("dbg_qt0", [128, N]), ("dbg_qt2", [128, N]),
                        ("dbg_se0", [128, 1024]), ("dbg_se1", [128, 1024]),
                        ("dbg_se16", [128, 1024]), ("dbg_at", [128, 3 * 1024]),
                        ("dbg_v", [128, 8 * NH * 128]), ("dbg_ln0", [128, NP])]:
            dbg_d[nm] = nc.declare_dram_parameter(nm, shp, F16, isOutput=True)

    with tile.TileContext(nc) as tc:
        with (
            tc.tile_pool(name="consts", bufs=1) as consts,
            tc.tile_pool(name="wpool", bufs=1) as wpool,
            tc.tile_pool(name="big", bufs=1) as bigp,
            tc.tile_pool(name="sexp", bufs=8) as sexp_p,
            tc.tile_pool(name="attn", bufs=2) as attn_p,
            tc.tile_pool(name="small", bufs=4) as small_p,
            tc.tile_pool(name="recp", bufs=4) as rec_p,
            tc.tile_pool(name="rbp", bufs=4) as rb_p,
            tc.tile_pool(name="outp", bufs=4) as out_p,
            tc.tile_pool(name="ps_s", bufs=2, space="PSUM") as ps_s,
            tc.tile_pool(name="ps_a", bufs=2, space="PSUM") as ps_a,
        ):
            # ---------------- DMA: x^T pieces + weights ----------------
            xTdp = [[bigp.tile([128, 1024], F16, name=f"xTd{i}_{sp}")
                     for sp in range(4)] for i in range(3)]

            def xtd_piece(sp):
                for ci, (c0, _r0, _rows) in enumerate(CCHUNKS):
                    nc.sync.dma_start(
                        out=xTdp[ci][sp],
                        in_=xdt_d[c0:c0 + 128, sp * 1024:(sp + 1) * 1024])

            wba1 = wpool.tile([128, 3840], F16, name="wba1")
            wba2 = wpool.tile([128, 960], F16, name="wba2")
            wbb = wpool.tile([128, 2880], F16, name="wbb")
            wb32 = wpool.tile([128, 963], F32, name="wb32")
            xtd_piece(0)
            nc.sync.dma_start(out=wba1, in_=wba_d[:, 0:3840])
            nc.sync.dma_start(out=wba2, in_=wba_d[:, 3840:4800])
            nc.sync.dma_start(out=wb32, in_=wb32_d[:, :])
            xtd_piece(1)
            nc.sync.dma_start(out=wbb, in_=wbb_d[:, :])
            xtd_piece(2)
            xtd_piece(3)

            srw_sb = [[wba1[:, (s * 3 + ci) * C:(s * 3 + ci + 1) * C]
                       for ci in range(3)] for s in range(4)]
            wq_sb = [wba2[:, ci * C:(ci + 1) * C] for ci in range(3)]
            wk_sb = [wbb[:, ci * C:(ci + 1) * C] for ci in range(3)]
            wv_sb = [wbb[:, (3 + ci) * C:(4 + ci) * C] for ci in range(3)]
            # wp chunk 2 has an extra ones-row (64) carrying bp
            wp_o = [wbb[0:128, (6 + 0) * C:(7 + 0) * C],
                    wbb[0:128, (6 + 1) * C:(7 + 1) * C],
                    wbb[0:65, (6 + 2) * C:(7 + 2) * C]]
            srb_bc = wb32[:, 0:C]
            bv_bc = wb32[:, C:2 * C]
            bk_col = [wb32[0:128, 3 * C + i:3 * C + i + 1] for i in range(3)]

            ident = consts.tile([128, 128], F16, name="ident")
            make_identity(nc, ident)
            eps_t = consts.tile([128, 1], F32, name="eps_t")
            nc.vector.memset(eps_t, LN_EPS)
            ebias_t = consts.tile([128, 1], F32, name="ebias_t")
            nc.vector.memset(ebias_t, EXP_BIAS)

            # warm the ACT exp table set during the ramp
            warm = small_p.tile([128, 1], F16, name="warm", tag="st")
            nc.scalar.activation(warm, eps_t,
                                 mybir.ActivationFunctionType.Exp)

            # v augmented: [128, kv_chunk(8), head(5), 128] with ones col 0
            v_aug = bigp.tile([128, 8, NH, 128], F16, name="v_aug")
            nc.vector.memset(v_aug[:, :, :, 0:64], 0.0)
            nc.vector.memset(v_aug[:, :, :, 0:1], 1.0)

            lnT = [bigp.tile([128, NP], F16, name=f"lnT{i}") for i in range(3)]
            # kT/qT: chunk 2 rows 64:128 duplicate rows 0:64 (head-4 pairing)
            kT = [bigp.tile([128, NP], F16, name=f"kT{i}") for i in range(3)]
            qT = [bigp.tile([128, N], F16, name=f"qT{i}") for i in range(3)]

            ln_tiles = [None] * 8
            ln_mid = [None] * 8

            # ---------------- prep building blocks ----------------
            def conv_group(it, s):
                if s == 0:
                    conv_group.pc = ps_a.tile([128, C], F32, name="pc", tag="a")
                pc = conv_group.pc
                t0 = it * 512 + s * 128
                sp, tc0 = t0 // 1024, t0 % 1024
                for ci, (_c0, r0, rows) in enumerate(CCHUNKS):
                    nc.tensor.matmul(pc, xTdp[ci][sp][r0:128, tc0:tc0 + 128],
                                     srw_sb[s][ci][r0:128, :],
                                     start=(s == 0 and ci == 0),
                                     stop=(s == 3 and ci == 2))
                if s == 3:
                    ln_stats(it, pc)

            def ln_stats(it, pc):
                cs = small_p.tile([128, C], F32, name="cs", tag="cvs", bufs=3)
                nc.vector.tensor_add(cs, pc, srb_bc)
                stats = small_p.tile([128, 6], F32, name="stats", tag="st")
                nc.vector.bn_stats(stats, cs)
                mv = small_p.tile([128, 2], F32, name="mv", tag="mv", bufs=3)
                nc.vector.bn_aggr(mv, stats)
                # rstd = rsqrt(var+eps) via Schraudolph seed + 1 Newton step
                s = small_p.tile([128, 8], F32, name="nrs", tag="nr", bufs=8)
                nc.vector.tensor_scalar_add(s[:, 0:1], mv[:, 1:2], LN_EPS)
                nc.vector.tensor_scalar(
                    s[:, 1:2].bitcast(I32), s[:, 0:1].bitcast(I32),
                    1, -1,
                    op0=mybir.AluOpType.logical_shift_right,
                    op1=mybir.AluOpType.bitwise_xor)
                nc.vector.tensor_scalar_add(
                    s[:, 2:3].bitcast(I32), s[:, 1:2].bitcast(I32),
                    0x5F3759DF + 1)
                y = s[:, 2:3]
                for c in (7,):
                    nc.vector.tensor_mul(s[:, 3:4], y, y)
                    nc.vector.tensor_mul(s[:, 5:6], s[:, 3:4], s[:, 0:1])
                    nc.vector.tensor_scalar(
                        s[:, 6:7], s[:, 5:6], -0.5, 1.5,
                        op0=mybir.AluOpType.mult, op1=mybir.AluOpType.add)
                    nc.vector.tensor_mul(s[:, c:c + 1], s[:, 6:7], y)
                    y = s[:, c:c + 1]
                ln_mid[it] = (cs, mv, y)

            def ln_finish(it):
                cs, mv, y = ln_mid[it]
                ln_h = small_p.tile([128, C], F16, name="ln_h", tag="lnf")
                nc.vector.tensor_scalar(ln_h, cs, mv[:, 0:1], y,
                                        op0=mybir.AluOpType.subtract,
                                        op1=mybir.AluOpType.mult)
                ln_tiles[it] = ln_h

            def emit_lnT(it):
                ln_h = ln_tiles[it]
                for ci, (c0, _r0, _rows) in enumerate(CCHUNKS):
                    pt = ps_s.tile([128, 128], F16, name="pt", tag="s")
                    nc.tensor.transpose(pt, ln_h[:, c0:c0 + 128], ident)
                    nc.scalar.copy(lnT[ci][:, it * 128:(it + 1) * 128], pt)

            def emit_v(it):
                pv = ps_s.tile([128, C], F32, name="pv", tag="s")
                for ci, (_c0, r0, rows) in enumerate(CCHUNKS):
                    nc.tensor.matmul(pv, lnT[ci][r0:128, it * 128:(it + 1) * 128],
                                     wv_sb[ci][r0:128, :],
                                     start=(ci == 0), stop=(ci == 2))
                nc.vector.tensor_add(
                    v_aug[:, it, :, 64:],
                    pv.rearrange("p (h d) -> p h d", h=NH),
                    bv_bc.rearrange("p (h d) -> p h d", h=NH))

            def emit_kT(i, b):
                """kT[i] columns [b*512, (b+1)*512)."""
                o0, osz = OCHUNKS[i]
                pk = ps_s.tile([128, QB], F32, name="pk", tag="s")
                for ci, (_c0, r0, rows) in enumerate(CCHUNKS):
                    nc.tensor.matmul(
                        pk[0:osz, :], wk_sb[ci][r0:128, o0:o0 + osz],
                        lnT[ci][r0:128, b * QB:(b + 1) * QB],
                        start=(ci == 0), stop=(ci == 2))
                nc.vector.tensor_scalar_add(
                    kT[i][0:osz, b * QB:(b + 1) * QB], pk[0:osz, :],
                    bk_col[i][0:osz, :])

            def emit_qproj(i, nb, on_act=False):
                o0, osz = OCHUNKS[i]
                pq = ps_s.tile([128, QB], F32, name="pq", tag="s")
                sp, tc0 = (nb * QB) // 1024, (nb * QB) % 1024
                for ci, (_c0, r0, rows) in enumerate(CCHUNKS):
                    nc.tensor.matmul(
                        pq[0:osz, :], wq_sb[ci][r0:128, o0:o0 + osz],
                        xTdp[ci][sp][r0:128, tc0:tc0 + QB],
                        start=(ci == 0), stop=(ci == 2))
                dst = qT[i][0:osz, nb * QB:(nb + 1) * QB]
                if on_act:
                    # ramp-time psum->sbuf copy rides the (exp-idle) ACT
                    nc.scalar.copy(dst, pq[0:osz, :])
                else:
                    nc.vector.tensor_copy(dst, pq[0:osz, :])

            # ---------------- attention ----------------
            # flat (qb, h, k) stream; scores(i) and av(i-2) interleave at
            # the matmul level (s,a,s,a) so every LDWEIGHTS hides behind
            # the previous matmul's stream.
            sdesc = [dict(qb=qb, h=h, k=k, first=(k == 0), last=(k == 7))
                     for qb in range(4) for h in range(NH) for k in range(8)]

            attnT = {}
            ses = {}
            pavs = {}
            norm2q = deque()
            prep = deque()

            def pump(n):
                for _ in range(n):
                    if prep:
                        prep.popleft()()

            def alloc_attnT(qb):
                ts = []
                for i, (_o0, osz) in enumerate(OCHUNKS):
                    rows = 65 if i == 2 else osz
                    t = attn_p.tile([rows, 1024], F16, name=f"aT{qb}_{i}",
                                    tag=f"attn{i}")
                    ts.append(t)
                nc.vector.memset(ts[2][64:65, :], 1.0)
                attnT[qb] = ts

            def emit_scores_mm(i, qh):
                d = sdesc[i]
                qb, h, k = d["qb"], d["h"], d["k"]
                ht, hr = h // 2, (h % 2) * 64 if h < 4 else 0
                if qh == 0:
                    ses[i] = (ps_s.tile([128, 1024], F32, name="ps", tag="s"),
                              sexp_p.tile([128, 1024], F16, name="se",
                                          tag="sexp"))
                ps, se = ses[i]
                q0 = qb * 1024 + qh * QB
                nc.tensor.matmul(
                    ps[:, qh * QB:(qh + 1) * QB],
                    kT[ht][hr:hr + HD, k * 128:(k + 1) * 128],
                    qT[ht][hr:hr + HD, q0:q0 + QB], start=True, stop=True)
                if qh == 1:
                    if i % DVE_EXP_MOD == DVE_EXP_OFF:
                        nc.vector.tensor_scalar(
                            se.bitcast(I16), ps, SCH_MUL, SCH_ADD,
                            op0=mybir.AluOpType.mult, op1=mybir.AluOpType.add)
                    else:
                        nc.scalar.activation(
                            se, ps, mybir.ActivationFunctionType.Exp,
                            bias=ebias_t, scale=SCALE)
                    if dbg and i in (0, 1, 16):
                        nm = {0: "dbg_se0", 1: "dbg_se1", 16: "dbg_se16"}[i]
                        nc.sync.dma_start(out=dbg_d[nm][:, :], in_=se)

            def emit_norm1(qb, h, pav):
                rec = rec_p.tile([1, 1024], F32, name="rec", tag="rc")
                nc.vector.reciprocal_approx_fast(rec, pav[0:1, :])
                rb = rb_p.tile([HD, 1024], F32, name="rb", tag="rb")
                nc.gpsimd.partition_broadcast(rb, rec)
                ci, dr = h // 2, (h % 2) * 64 if h < 4 else 0
                norm2q.append((qb, ci, dr, pav, rb))

            def flush_norm2():
                while norm2q:
                    qb, ci, dr, pav, rb = norm2q.popleft()
                    nc.vector.tensor_mul(
                        attnT[qb][ci][dr:dr + HD, :], pav[64:128, :], rb)

            def emit_av_mm(i, qh):
                d = sdesc[i]
                qb, h, k = d["qb"], d["h"], d["k"]
                if d["first"] and qh == 0:
                    pavs[i - i % 8] = ps_a.tile([128, 1024], F32, name="pav",
                                                tag="a")
                pav = pavs[i - i % 8]
                _ps, se = ses[i]
                nc.tensor.matmul(
                    pav[:, qh * QB:(qh + 1) * QB], v_aug[:, k, h, :],
                    se[:, qh * QB:(qh + 1) * QB],
                    start=d["first"], stop=d["last"])
                if qh == 1:
                    del ses[i]
                    if d["last"]:
                        emit_norm1(qb, h, pavs.pop(i - i % 8))
                        if h == NH - 1:
                            for qs in range(8):
                                prep.append(
                                    lambda qb=qb, qs=qs: emit_proj_qs(qb, qs))

            def emit_proj_qs(qb, qs):
                po = ps_a.tile([128, C], F32, name="po", tag="a")
                for ci in range(3):
                    rows = 65 if ci == 2 else OCHUNKS[ci][1]
                    nc.tensor.matmul(
                        po, attnT[qb][ci][0:rows, qs * 128:(qs + 1) * 128],
                        wp_o[ci], start=(ci == 0), stop=(ci == 2))
                o_sb = out_p.tile([128, C], F16, name="o_sb", tag="o")
                nc.vector.tensor_copy(o_sb, po)
                g = qb * 8 + qs
                nc.sync.dma_start(out=out_d[g * 128:(g + 1) * 128, :],
                                  in_=o_sb)

            # ---------------- ramp ----------------
            warm_ps = ps_s.tile([128, 128], F32, name="warm_ps", tag="s")
            for _ in range(160):
                nc.tensor.matmul(warm_ps, ident, ident, start=True, stop=True)
            for it in range(8):
                for s in range(4):
                    conv_group(it, s)
                if it >= 1:
                    ln_finish(it - 1)
                    emit_lnT(it - 1)
            ln_finish(7)
            emit_lnT(7)
            emit_kT(0, 0)
            emit_kT(1, 0)
            emit_kT(2, 0)
            for i in range(3):
                emit_qproj(i, 0, on_act=True)
                emit_qproj(i, 1, on_act=True)
            emit_v(0)

            # remaining prep drip-fed into PE slack during attention
            for it in range(1, 8):
                prep.append(lambda it=it: emit_v(it))
            for i in range(3):
                prep.append(lambda i=i: emit_kT(i, 1))
            for nb in range(2, 8):
                for i in range(3):
                    prep.append(lambda i=i, nb=nb: emit_qproj(i, nb))

            # flat lag-2 stream, interleaved at the matmul level:
            # s(i,qh0) a(i-2,qh0) s(i,qh1) a(i-2,qh1)
            for i in range(len(sdesc)):
                d = sdesc[i]
                if d["h"] == 0 and d["first"]:
                    alloc_attnT(d["qb"])
                emit_scores_mm(i, 0)
                if i >= 2:
                    emit_av_mm(i - 2, 0)
                emit_scores_mm(i, 1)
                if i >= 2:
                    emit_av_mm(i - 2, 1)
                flush_norm2()
                pump(2 if i < 12 else 1)
            for j in (len(sdesc) - 2, len(sdesc) - 1):
                emit_av_mm(j, 0)
                emit_av_mm(j, 1)
                flush_norm2()
            pump(len(prep))
            flush_norm2()
            if dbg:
                nc.sync.dma_start(out=dbg_d["dbg_kt0"][:, :], in_=kT[0])
                nc.sync.dma_start(out=dbg_d["dbg_kt2"][:, :], in_=kT[2])
                nc.sync.dma_start(out=dbg_d["dbg_qt0"][:, :], in_=qT[0])
                nc.sync.dma_start(out=dbg_d["dbg_qt2"][:, :], in_=qT[2])
                nc.sync.dma_start(out=dbg_d["dbg_ln0"][:, :], in_=lnT[0])
                nc.sync.dma_start(
                    out=dbg_d["dbg_v"][:, :],
                    in_=v_aug.rearrange("p a b c -> p (a b c)"))
                for ci in range(3):
                    nc.sync.dma_start(
                        out=dbg_d["dbg_at"][0:OCHUNKS[ci][1],
                                            ci * 1024:(ci + 1) * 1024],
                        in_=attnT[3][ci][0:OCHUNKS[ci][1], :])

    nc.compile()
    return nc


_CACHE = {}


def _get_nc():
    if "nc" not in _CACHE:
        _CACHE["nc"] = build_bass()
    return _CACHE["nc"]


def make_in_maps(x, Wq, Wkv, sr_w, sr_b, ln_g, ln_b, Wp, bp):
    B = x.shape[0]
    f16 = np.float16
    f32 = np.float32
    ln_g = np.asarray(ln_g, f32)
    ln_b = np.asarray(ln_b, f32)
    wk_f = np.asarray(Wkv[:, :C], f32)
    wv_f = np.asarray(Wkv[:, C:], f32)
    wq = np.ascontiguousarray(Wq, dtype=f16)
    # fold LN gamma/beta into the K/V projections
    wk = np.ascontiguousarray(ln_g[:, None] * wk_f, dtype=f16)
    wv = np.ascontiguousarray(ln_g[:, None] * wv_f, dtype=f16)
    bk = np.ascontiguousarray(ln_b @ wk_f, dtype=f32)
    bv = np.ascontiguousarray(ln_b @ wv_f, dtype=f32)
    srw = np.ascontiguousarray(np.asarray(sr_w, dtype=f16).reshape(4 * C, C))
    wp = np.ascontiguousarray(Wp, dtype=f16)
    srb = np.ascontiguousarray(sr_b, dtype=f32)
    bpv = np.ascontiguousarray(bp, dtype=f32)
    xf = np.asarray(x, dtype=f16)
    xdt = np.ascontiguousarray(
        xf.reshape(B, 8, 4, 2, 32, 2, C)         # [B, it, h'lo, dh, w', dw, C]
          .transpose(0, 6, 1, 3, 5, 2, 4)         # [B, C, it, dh, dw, h'lo, w']
          .reshape(B, C, N))
    CH = [(0, 0), (128, 0), (192, 64)]            # (c0, r0)
    wba = np.zeros((128, 4800), f16)
    for s in range(4):
        for ci, (c0, r0) in enumerate(CH):
            col = (s * 3 + ci) * C
            wba[r0:128, col:col + C] = srw[s * C + c0 + r0:s * C + c0 + 128, :]
    for ci, (c0, r0) in enumerate(CH):
        wba[:, (12 + ci) * C:(13 + ci) * C] = wq[c0:c0 + 128, :]
    wbb = np.zeros((128, 2880), f16)
    for ci, (c0, r0) in enumerate(CH):
        wbb[:, ci * C:(ci + 1) * C] = wk[c0:c0 + 128, :]
        wbb[:, (3 + ci) * C:(4 + ci) * C] = wv[c0:c0 + 128, :]
    OCH = [(0, 128), (128, 128), (256, 64)]
    for i, (o0, osz) in enumerate(OCH):
        wbb[0:osz, (6 + i) * C:(7 + i) * C] = wp[o0:o0 + osz, :]
    # proj bias rides a ones-row in attnT[2] against this bp row
    wbb[64, (6 + 2) * C:(7 + 2) * C] = bpv
    wb32 = np.zeros((128, 963), f32)
    wb32[:, 0:C] = srb[None, :]
    wb32[:, C:2 * C] = bv[None, :]
    wb32[:, 2 * C:3 * C] = bpv[None, :]
    for i, (o0, osz) in enumerate(OCH):
        wb32[0:osz, 3 * C + i] = bk[o0:o0 + osz]
    # kT[2] is row-duplicated -> duplicate its bias column too
    wb32[64:128, 3 * C + 2] = bk[256:320]
    wba = np.ascontiguousarray(wba)
    wbb = np.ascontiguousarray(wbb)
    wb32 = np.ascontiguousarray(wb32)
    return [
        {"xdt": xdt[i], "wba": wba, "wbb": wbb, "wb32": wb32}
        for i in range(B)
    ]


def _xd_to_orig_rows():
    """orig token row for each xd-order row (device output ordering)."""
    idx = np.arange(N)
    it, s, l = idx >> 9, (idx >> 7) & 3, idx & 127
    dh, dw = s >> 1, s & 1
    return (8 * it + dh + 2 * (l >> 5)) * 64 + 2 * (l & 31) + dw


_ORIG_ROWS = _xd_to_orig_rows()


def kernel(x, Wq, Wkv, sr_w, sr_b, ln_g, ln_b, Wp, bp, H=64, W=64):
    x = np.asarray(x, dtype=np.float32)
    B = x.shape[0]
    assert x.shape == (B, N, C), x.shape
    nc = _get_nc()
    in_maps = make_in_maps(x, Wq, Wkv, sr_w, sr_b, ln_g, ln_b, Wp, bp)
    res = run_bass_kernel_spmd(nc, in_maps, core_ids=list(range(8)))
    out = np.empty((B, N, C), np.float32)
    for i in range(B):
        out[i, _ORIG_ROWS, :] = np.asarray(res.results[i]["out"], np.float32)
    return out


# revision 24
# speedup vs baseline: 1.0952x; 1.0952x over previous
"""PVT-style spatial-reduction attention on 8 Trainium2 NeuronCores.

Sharding: data-parallel over batch (B=8 -> one batch element per core).
Each core runs the full attention for its batch element; weights are
replicated. No collectives needed.

v3 schedule (v2 was ACT-exp-limited at ~91% PE busy; this version attacks
both PE cycles and the elementwise wall):
  - scores matmuls have K=hd=64, so the two heads of a pair run
    CONCURRENTLY in the PE array via row tiling (tile_position (0,0) and
    (64,0) auto-derived from base partitions) -> ~2x on the score phase.
    Head 4 pairs its own kv-chunks against row-duplicated qT[2]/kT[2]
    (the duplicates are written by col-tiled double matmuls, ~free).
  - exp: most steps on ACT ([128,1024] ACTIVATE per head); a fraction on
    DVE as a single tensor_scalar Schraudolph exp straight into f16 bits
    (i16 = round(184.665*score + 10868.74); +-3% per weight, cancels in
    softmax; validated 1e-2 rel err at 100% usage, we use ~30%).
  - softmax normalization: denominators ride the av matmul (ones column),
    one reciprocal [1,1024] + gpsimd partition_broadcast + ONE DVE
    multiply reading the av psum directly (no staging copy).
  - output projection bias is folded in via a ones row in attnT[2] and a
    bp row in the Wp blob; qproj psum->sbuf copies ride the (ramp-idle)
    ACT engine.
  - PSUM: pool "s" = scores/prep ring (2x[128,1024]f32 = 4 banks),
    pool "a" = conv/av/proj ring (2x[128,1024]f32 = 4 banks).
"""

import os
import sys
from collections import deque

import numpy as np

for _p in ("/opt/trn_rl_repo", "/root/.axon_site/_ro/trn_rl_repo"):
    if os.path.isdir(_p) and _p not in sys.path:
        sys.path.append(_p)

import concourse.bacc as bacc
import concourse.bass as bass
import concourse.mybir as mybir
import concourse.tile as tile
from concourse.bass_utils import run_bass_kernel_spmd
from concourse.masks import make_identity

F16 = mybir.dt.float16
F32 = mybir.dt.float32
I16 = mybir.dt.int16
I32 = mybir.dt.int32

N = 4096          # q tokens (H*W = 64*64)
C = 320           # model dim
NH = 5            # heads
HD = 64           # head dim
NP = 1024         # kv tokens ((H/2)*(W/2))
QB = 512
LN_EPS = 1e-3
SCALE = HD ** -0.5
# NOTE: bias must be 0: raw scores reach +-65, and a negative Schraudolph
# int16 bitcasts to a LARGE negative f16 (not a tiny one), nuking the
# softmax. With bias 0 the i16 range is [3187, 27305] - safe both ends.
EXP_BIAS = 0.0

# Schraudolph f16 exp: i16 = round(A*(SCALE*x+EXP_BIAS) + (15360-59.3))
SCH_A = 1477.3195
SCH_MUL = SCH_A * SCALE
SCH_ADD = 15360.0 - 59.3 + SCH_A * EXP_BIAS

# contraction chunks over C=320: three 128-row tiles; the last one holds
# c 192:320 and uses rows 64:128 (its top 64 rows overlap chunk 1).
CCHUNKS = [(0, 0, 128), (128, 0, 128), (192, 64, 128)]  # (c_start, row0, rows)
# output chunks over C=320
OCHUNKS = [(0, 128), (128, 128), (256, 64)]

# which attention steps compute exp on DVE (Schraudolph) instead of ACT.
# Only k==5: at that point in each (qb,h) unit the DVE queue is shallow
# (norm muls flushed at k~1, proj copies delayed), so the exp is picked up
# promptly and the scores-ring WAR does not stall the PE. (v3 routed every
# 4th step through a busy DVE FIFO; the 2-4us latency stalls tripped HAM
# into 1.2GHz windows.)
DVE_EXP_K = 5


def build_bass(dbg=False):
    nc = bacc.Bacc("TRN2", target_bir_lowering=False, debug=False, num_devices=8)

    xdt_d = nc.declare_dram_parameter("xdt", [C, N], F16, isOutput=False)
    wba_d = nc.declare_dram_parameter("wba", [128, 4800], F16, isOutput=False)
    wbb_d = nc.declare_dram_parameter("wbb", [128, 2880], F16, isOutput=False)
    wb32_d = nc.declare_dram_parameter("wb32", [128, 963], F32, isOutput=False)
    out_d = nc.declare_dram_parameter("out", [N, C], F16, isOutput=True)
    dbg_d = {}
    if dbg:
        for nm, shp in [("dbg_kt0", [128, NP]), ("dbg_kt2", [128, NP]),
                        ("dbg_qt0", [128, N]), ("dbg_qt2", [128, N]),
                        ("dbg_se0", [128, 1024]), ("dbg_se1", [128, 1024]),
                        ("dbg_se16", [128, 1024]), ("dbg_at", [128, 3 * 1024]),
                        ("dbg_v", [128, 8 * NH * 128]), ("dbg_ln0", [128, NP])]:
            dbg_d[nm] = nc.declare_dram_parameter(nm, shp, F16, isOutput=True)

    with tile.TileContext(nc) as tc:
        with (
            tc.tile_pool(name="consts", bufs=1) as consts,
            tc.tile_pool(name="wpool", bufs=1) as wpool,
            tc.tile_pool(name="big", bufs=1) as bigp,
            tc.tile_pool(name="sexp", bufs=8) as sexp_p,
            tc.tile_pool(name="attn", bufs=2) as attn_p,
            tc.tile_pool(name="small", bufs=4) as small_p,
            tc.tile_pool(name="recp", bufs=4) as rec_p,
            tc.tile_pool(name="rbp", bufs=4) as rb_p,
            tc.tile_pool(name="outp", bufs=4) as out_p,
            tc.tile_pool(name="ps_s", bufs=2, space="PSUM") as ps_s,
            tc.tile_pool(name="ps_a", bufs=2, space="PSUM") as ps_a,
        ):
            # ---------------- DMA: x^T pieces + weights ----------------
            xTdp = [[bigp.tile([128, 1024], F16, name=f"xTd{i}_{sp}")
                     for sp in range(4)] for i in range(3)]

            def xtd_piece(sp):
                for ci, (c0, _r0, _rows) in enumerate(CCHUNKS):
                    nc.sync.dma_start(
                        out=xTdp[ci][sp],
                        in_=xdt_d[c0:c0 + 128, sp * 1024:(sp + 1) * 1024])

            wba1 = wpool.tile([128, 3840], F16, name="wba1")
            wba2 = wpool.tile([128, 960], F16, name="wba2")
            wbb = wpool.tile([128, 2880], F16, name="wbb")
            wb32 = wpool.tile([128, 963], F32, name="wb32")
            xtd_piece(0)
            nc.sync.dma_start(out=wba1, in_=wba_d[:, 0:3840])
            nc.sync.dma_start(out=wba2, in_=wba_d[:, 3840:4800])
            nc.sync.dma_start(out=wb32, in_=wb32_d[:, :])
            xtd_piece(1)
            nc.sync.dma_start(out=wbb, in_=wbb_d[:, :])
            xtd_piece(2)
            xtd_piece(3)

            srw_sb = [[wba1[:, (s * 3 + ci) * C:(s * 3 + ci + 1) * C]
                       for ci in range(3)] for s in range(4)]
            wq_sb = [wba2[:, ci * C:(ci + 1) * C] for ci in range(3)]
            wk_sb = [wbb[:, ci * C:(ci + 1) * C] for ci in range(3)]
            wv_sb = [wbb[:, (3 + ci) * C:(4 + ci) * C] for ci in range(3)]
            # wp chunk 2 has an extra ones-row (64) carrying bp
            wp_o = [wbb[0:128, (6 + 0) * C:(7 + 0) * C],
                    wbb[0:128, (6 + 1) * C:(7 + 1) * C],
                    wbb[0:65, (6 + 2) * C:(7 + 2) * C]]
            srb_bc = wb32[:, 0:C]
            bv_bc = wb32[:, C:2 * C]
            bk_col = [wb32[0:128, 3 * C + i:3 * C + i + 1] for i in range(3)]

            ident = consts.tile([128, 128], F16, name="ident")
            make_identity(nc, ident)
            eps_t = consts.tile([128, 1], F32, name="eps_t")
            nc.vector.memset(eps_t, LN_EPS)
            ebias_t = consts.tile([128, 1], F32, name="ebias_t")
            nc.vector.memset(ebias_t, EXP_BIAS)

            # warm the ACT exp table set during the ramp
            warm = small_p.tile([128, 1], F16, name="warm", tag="st")
            nc.scalar.activation(warm, eps_t,
                                 mybir.ActivationFunctionType.Exp)

            # v augmented: [128, kv_chunk(8), head(5), 128] with ones col 0
            v_aug = bigp.tile([128, 8, NH, 128], F16, name="v_aug")
            nc.vector.memset(v_aug[:, :, :, 0:64], 0.0)
            nc.vector.memset(v_aug[:, :, :, 0:1], 1.0)

            lnT = [bigp.tile([128, NP], F16, name=f"lnT{i}") for i in range(3)]
            # kT/qT: chunk 2 rows 64:128 duplicate rows 0:64 (head-4 pairing)
            kT = [bigp.tile([128, NP], F16, name=f"kT{i}") for i in range(3)]
            qT = [bigp.tile([128, N], F16, name=f"qT{i}") for i in range(3)]

            ln_tiles = [None] * 8
            ln_mid = [None] * 8

            # ---------------- prep building blocks ----------------
            def conv_group(it, s):
                if s == 0:
                    conv_group.pc = ps_a.tile([128, C], F32, name="pc", tag="a")
                pc = conv_group.pc
                t0 = it * 512 + s * 128
                sp, tc0 = t0 // 1024, t0 % 1024
                for ci, (_c0, r0, rows) in enumerate(CCHUNKS):
                    nc.tensor.matmul(pc, xTdp[ci][sp][r0:128, tc0:tc0 + 128],
                                     srw_sb[s][ci][r0:128, :],
                                     start=(s == 0 and ci == 0),
                                     stop=(s == 3 and ci == 2))
                if s == 3:
                    ln_stats(it, pc)

            def ln_stats(it, pc):
                cs = small_p.tile([128, C], F32, name="cs", tag="cvs", bufs=3)
                nc.vector.tensor_add(cs, pc, srb_bc)
                stats = small_p.tile([128, 6], F32, name="stats", tag="st")
                nc.vector.bn_stats(stats, cs)
                mv = small_p.tile([128, 2], F32, name="mv", tag="mv", bufs=3)
                nc.vector.bn_aggr(mv, stats)
                # rstd = rsqrt(var+eps) via Schraudolph seed + 1 Newton step
                s = small_p.tile([128, 8], F32, name="nrs", tag="nr", bufs=8)
                nc.vector.tensor_scalar_add(s[:, 0:1], mv[:, 1:2], LN_EPS)
                nc.vector.tensor_scalar(
                    s[:, 1:2].bitcast(I32), s[:, 0:1].bitcast(I32),
                    1, -1,
                    op0=mybir.AluOpType.logical_shift_right,
                    op1=mybir.AluOpType.bitwise_xor)
                nc.vector.tensor_scalar_add(
                    s[:, 2:3].bitcast(I32), s[:, 1:2].bitcast(I32),
                    0x5F3759DF + 1)
                y = s[:, 2:3]
                for c in (7,):
                    nc.vector.tensor_mul(s[:, 3:4], y, y)
                    nc.vector.tensor_mul(s[:, 5:6], s[:, 3:4], s[:, 0:1])
                    nc.vector.tensor_scalar(
                        s[:, 6:7], s[:, 5:6], -0.5, 1.5,
                        op0=mybir.AluOpType.mult, op1=mybir.AluOpType.add)
                    nc.vector.tensor_mul(s[:, c:c + 1], s[:, 6:7], y)
                    y = s[:, c:c + 1]
                ln_mid[it] = (cs, mv, y)

            def ln_finish(it):
                cs, mv, y = ln_mid[it]
                ln_h = small_p.tile([128, C], F16, name="ln_h", tag="lnf")
                nc.vector.tensor_scalar(ln_h, cs, mv[:, 0:1], y,
                                        op0=mybir.AluOpType.subtract,
                                        op1=mybir.AluOpType.mult)
                ln_tiles[it] = ln_h

            def emit_lnT(it):
                ln_h = ln_tiles[it]
                for ci, (c0, _r0, _rows) in enumerate(CCHUNKS):
                    pt = ps_s.tile([128, 128], F16, name="pt", tag="s")
                    nc.tensor.transpose(pt, ln_h[:, c0:c0 + 128], ident)
                    nc.scalar.copy(lnT[ci][:, it * 128:(it + 1) * 128], pt)

            def emit_v(it):
                pv = ps_s.tile([128, C], F32, name="pv", tag="s")
                for ci, (_c0, r0, rows) in enumerate(CCHUNKS):
                    nc.tensor.matmul(pv, lnT[ci][r0:128, it * 128:(it + 1) * 128],
                                     wv_sb[ci][r0:128, :],
                                     start=(ci == 0), stop=(ci == 2))
                nc.vector.tensor_add(
                    v_aug[:, it, :, 64:],
                    pv.rearrange("p (h d) -> p h d", h=NH),
                    bv_bc.rearrange("p (h d) -> p h d", h=NH))

            def emit_kT(i, b):
                """kT[i] columns [b*512, (b+1)*512)."""
                o0, osz = OCHUNKS[i]
                pk = ps_s.tile([128, QB], F32, name="pk", tag="s")
                for ci, (_c0, r0, rows) in enumerate(CCHUNKS):
                    nc.tensor.matmul(
                        pk[0:osz, :], wk_sb[ci][r0:128, o0:o0 + osz],
                        lnT[ci][r0:128, b * QB:(b + 1) * QB],
                        start=(ci == 0), stop=(ci == 2))
                nc.vector.tensor_scalar_add(
                    kT[i][0:osz, b * QB:(b + 1) * QB], pk[0:osz, :],
                    bk_col[i][0:osz, :])

            def emit_qproj(i, nb, on_act=False):
                o0, osz = OCHUNKS[i]
                pq = ps_s.tile([128, QB], F32, name="pq", tag="s")
                sp, tc0 = (nb * QB) // 1024, (nb * QB) % 1024
                for ci, (_c0, r0, rows) in enumerate(CCHUNKS):
                    nc.tensor.matmul(
                        pq[0:osz, :], wq_sb[ci][r0:128, o0:o0 + osz],
                        xTdp[ci][sp][r0:128, tc0:tc0 + QB],
                        start=(ci == 0), stop=(ci == 2))
                dst = qT[i][0:osz, nb * QB:(nb + 1) * QB]
                if on_act:
                    # ramp-time psum->sbuf copy rides the (exp-idle) ACT
                    nc.scalar.copy(dst, pq[0:osz, :])
                else:
                    nc.vector.tensor_copy(dst, pq[0:osz, :])

            # ---------------- attention ----------------
            # flat (qb, h, k) stream; scores(i) and av(i-2) interleave at
            # the matmul level (s,a,s,a) so every LDWEIGHTS hides behind
            # the previous matmul's stream.
            sdesc = [dict(qb=qb, h=h, k=k, first=(k == 0), last=(k == 7))
                     for qb in range(4) for h in range(NH) for k in range(8)]

            attnT = {}
            ses = {}
            pavs = {}
            norm2q = deque()
            prep = deque()
            prep_later = deque()   # (ready_step, fn)

            def pump(n, i=1 << 30):
                while prep_later and prep_later[0][0] <= i:
                    prep.append(prep_later.popleft()[1])
                for _ in range(n):
                    if prep:
                        prep.popleft()()

            def alloc_attnT(qb):
                ts = []
                for i, (_o0, osz) in enumerate(OCHUNKS):
                    rows = 65 if i == 2 else osz
                    t = attn_p.tile([rows, 1024], F16, name=f"aT{qb}_{i}",
                                    tag=f"attn{i}")
                    ts.append(t)
                nc.vector.memset(ts[2][64:65, :], 1.0)
                attnT[qb] = ts

            def emit_scores_mm(i, qh):
                d = sdesc[i]
                qb, h, k = d["qb"], d["h"], d["k"]
                ht, hr = h // 2, (h % 2) * 64 if h < 4 else 0
                if qh == 0:
                    ses[i] = (ps_s.tile([128, 1024], F32, name="ps", tag="s"),
                              sexp_p.tile([128, 1024], F16, name="se",
                                          tag="sexp"))
                ps, se = ses[i]
                q0 = qb * 1024 + qh * QB
                nc.tensor.matmul(
                    ps[:, qh * QB:(qh + 1) * QB],
                    kT[ht][hr:hr + HD, k * 128:(k + 1) * 128],
                    qT[ht][hr:hr + HD, q0:q0 + QB], start=True, stop=True)
                if qh == 1:
                    if d["k"] == DVE_EXP_K:
                        nc.vector.tensor_scalar(
                            se.bitcast(I16), ps, SCH_MUL, SCH_ADD,
                            op0=mybir.AluOpType.mult, op1=mybir.AluOpType.add)
                    else:
                        nc.scalar.activation(
                            se, ps, mybir.ActivationFunctionType.Exp,
                            bias=ebias_t, scale=SCALE)
                    if dbg and i in (0, 1, 16):
                        nm = {0: "dbg_se0", 1: "dbg_se1", 16: "dbg_se16"}[i]
                        nc.sync.dma_start(out=dbg_d[nm][:, :], in_=se)

            def emit_norm1(qb, h, pav):
                rec = rec_p.tile([1, 1024], F32, name="rec", tag="rc")
                nc.vector.reciprocal_approx_fast(rec, pav[0:1, :])
                rb = rb_p.tile([HD, 1024], F32, name="rb", tag="rb")
                nc.gpsimd.partition_broadcast(rb, rec)
                ci, dr = h // 2, (h % 2) * 64 if h < 4 else 0
                norm2q.append((qb, ci, dr, pav, rb))

            def flush_norm2():
                while norm2q:
                    qb, ci, dr, pav, rb = norm2q.popleft()
                    nc.vector.tensor_mul(
                        attnT[qb][ci][dr:dr + HD, :], pav[64:128, :], rb)

            def emit_av_mm(i, qh):
                d = sdesc[i]
                qb, h, k = d["qb"], d["h"], d["k"]
                if d["first"] and qh == 0:
                    pavs[i - i % 8] = ps_a.tile([128, 1024], F32, name="pav",
                                                tag="a")
                pav = pavs[i - i % 8]
                _ps, se = ses[i]
                nc.tensor.matmul(
                    pav[:, qh * QB:(qh + 1) * QB], v_aug[:, k, h, :],
                    se[:, qh * QB:(qh + 1) * QB],
                    start=d["first"], stop=d["last"])
                if qh == 1:
                    del ses[i]
                    if d["last"]:
                        emit_norm1(qb, h, pavs.pop(i - i % 8))
                        if h == NH - 1:
                            # delay proj: the first proj matmul waits on the
                            # last head's norm mul, and an in-order PE queue
                            # would stall ~1.7us on it if pumped immediately
                            for qs in range(8):
                                prep_later.append(
                                    (i + 4,
                                     lambda qb=qb, qs=qs: emit_proj_qs(qb, qs)))

            def emit_proj_qs(qb, qs):
                po = ps_a.tile([128, C], F32, name="po", tag="a")
                for ci in range(3):
                    rows = 65 if ci == 2 else OCHUNKS[ci][1]
                    nc.tensor.matmul(
                        po, attnT[qb][ci][0:rows, qs * 128:(qs + 1) * 128],
                        wp_o[ci], start=(ci == 0), stop=(ci == 2))
                o_sb = out_p.tile([128, C], F16, name="o_sb", tag="o")
                nc.vector.tensor_copy(o_sb, po)
                g = qb * 8 + qs
                nc.sync.dma_start(out=out_d[g * 128:(g + 1) * 128, :],
                                  in_=o_sb)

            # ---------------- ramp ----------------
            warm_ps = ps_s.tile([128, 128], F32, name="warm_ps", tag="s")
            for _ in range(160):
                nc.tensor.matmul(warm_ps, ident, ident, start=True, stop=True)
            for it in range(8):
                for s in range(4):
                    conv_group(it, s)
                if it >= 1:
                    ln_finish(it - 1)
                    emit_lnT(it - 1)
            ln_finish(7)
            emit_lnT(7)
            emit_kT(0, 0)
            emit_kT(1, 0)
            emit_kT(2, 0)
            for i in range(3):
                emit_qproj(i, 0, on_act=True)
                emit_qproj(i, 1, on_act=True)
            emit_v(0)

            # remaining prep drip-fed into PE slack during attention
            for it in range(1, 8):
                prep.append(lambda it=it: emit_v(it))
            for i in range(3):
                prep.append(lambda i=i: emit_kT(i, 1))
            for nb in range(2, 8):
                for i in range(3):
                    prep.append(lambda i=i, nb=nb: emit_qproj(i, nb))

            # flat lag-2 stream: s,s then a,a per step (consecutive same-
            # stationary matmuls avoid the ~87ns LDWEIGHTS exposure that an
            # alternating order pays on every matmul)
            for i in range(len(sdesc)):
                d = sdesc[i]
                if d["h"] == 0 and d["first"]:
                    alloc_attnT(d["qb"])
                emit_scores_mm(i, 0)
                emit_scores_mm(i, 1)
                if i >= 2:
                    emit_av_mm(i - 2, 0)
                    emit_av_mm(i - 2, 1)
                flush_norm2()
                pump(2 if i < 12 else 1, i)
            for j in (len(sdesc) - 2, len(sdesc) - 1):
                emit_av_mm(j, 0)
                emit_av_mm(j, 1)
                flush_norm2()
            pump(len(prep) + len(prep_later))
            flush_norm2()
            if dbg:
                nc.sync.dma_start(out=dbg_d["dbg_kt0"][:, :], in_=kT[0])
                nc.sync.dma_start(out=dbg_d["dbg_kt2"][:, :], in_=kT[2])
                nc.sync.dma_start(out=dbg_d["dbg_qt0"][:, :], in_=qT[0])
                nc.sync.dma_start(out=dbg_d["dbg_qt2"][:, :], in_=qT[2])
                nc.sync.dma_start(out=dbg_d["dbg_ln0"][:, :], in_=lnT[0])
                nc.sync.dma_start(
                    out=dbg_d["dbg_v"][:, :],
                    in_=v_aug.rearrange("p a b c -> p (a b c)"))
                for ci in range(3):
                    nc.sync.dma_start(
                        out=dbg_d["dbg_at"][0:OCHUNKS[ci][1],
                                            ci * 1024:(ci + 1) * 1024],
                        in_=attnT[3][ci][0:OCHUNKS[ci][1], :])

    nc.compile()
    return nc


_CACHE = {}


def _get_nc():
    if "nc" not in _CACHE:
        _CACHE["nc"] = build_bass()
    return _CACHE["nc"]


def make_in_maps(x, Wq, Wkv, sr_w, sr_b, ln_g, ln_b, Wp, bp):
    B = x.shape[0]
    f16 = np.float16
    f32 = np.float32
    ln_g = np.asarray(ln_g, f32)
    ln_b = np.asarray(ln_b, f32)
    wk_f = np.asarray(Wkv[:, :C], f32)
    wv_f = np.asarray(Wkv[:, C:], f32)
    wq = np.ascontiguousarray(Wq, dtype=f16)
    # fold LN gamma/beta into the K/V projections
    wk = np.ascontiguousarray(ln_g[:, None] * wk_f, dtype=f16)
    wv = np.ascontiguousarray(ln_g[:, None] * wv_f, dtype=f16)
    bk = np.ascontiguousarray(ln_b @ wk_f, dtype=f32)
    bv = np.ascontiguousarray(ln_b @ wv_f, dtype=f32)
    srw = np.ascontiguousarray(np.asarray(sr_w, dtype=f16).reshape(4 * C, C))
    wp = np.ascontiguousarray(Wp, dtype=f16)
    srb = np.ascontiguousarray(sr_b, dtype=f32)
    bpv = np.ascontiguousarray(bp, dtype=f32)
    xf = np.asarray(x, dtype=f16)
    xdt = np.ascontiguousarray(
        xf.reshape(B, 8, 4, 2, 32, 2, C)         # [B, it, h'lo, dh, w', dw, C]
          .transpose(0, 6, 1, 3, 5, 2, 4)         # [B, C, it, dh, dw, h'lo, w']
          .reshape(B, C, N))
    CH = [(0, 0), (128, 0), (192, 64)]            # (c0, r0)
    wba = np.zeros((128, 4800), f16)
    for s in range(4):
        for ci, (c0, r0) in enumerate(CH):
            col = (s * 3 + ci) * C
            wba[r0:128, col:col + C] = srw[s * C + c0 + r0:s * C + c0 + 128, :]
    for ci, (c0, r0) in enumerate(CH):
        wba[:, (12 + ci) * C:(13 + ci) * C] = wq[c0:c0 + 128, :]
    wbb = np.zeros((128, 2880), f16)
    for ci, (c0, r0) in enumerate(CH):
        wbb[:, ci * C:(ci + 1) * C] = wk[c0:c0 + 128, :]
        wbb[:, (3 + ci) * C:(4 + ci) * C] = wv[c0:c0 + 128, :]
    OCH = [(0, 128), (128, 128), (256, 64)]
    for i, (o0, osz) in enumerate(OCH):
        wbb[0:osz, (6 + i) * C:(7 + i) * C] = wp[o0:o0 + osz, :]
    # proj bias rides a ones-row in attnT[2] against this bp row
    wbb[64, (6 + 2) * C:(7 + 2) * C] = bpv
    wb32 = np.zeros((128, 963), f32)
    wb32[:, 0:C] = srb[None, :]
    wb32[:, C:2 * C] = bv[None, :]
    wb32[:, 2 * C:3 * C] = bpv[None, :]
    for i, (o0, osz) in enumerate(OCH):
        wb32[0:osz, 3 * C + i] = bk[o0:o0 + osz]
    # kT[2] is row-duplicated -> duplicate its bias column too
    wb32[64:128, 3 * C + 2] = bk[256:320]
    wba = np.ascontiguousarray(wba)
    wbb = np.ascontiguousarray(wbb)
    wb32 = np.ascontiguousarray(wb32)
    return [
        {"xdt": xdt[i], "wba": wba, "wbb": wbb, "wb32": wb32}
        for i in range(B)
    ]


def _xd_to_orig_rows():
    """orig token row for each xd-order row (device output ordering)."""
    idx = np.arange(N)
    it, s, l = idx >> 9, (idx >> 7) & 3, idx & 127
    dh, dw = s >> 1, s & 1
    return (8 * it + dh + 2 * (l >> 5)) * 64 + 2 * (l & 31) + dw


_ORIG_ROWS = _xd_to_orig_rows()


def kernel(x, Wq, Wkv, sr_w, sr_b, ln_g, ln_b, Wp, bp, H=64, W=64):
    x = np.asarray(x, dtype=np.float32)
    B = x.shape[0]
    assert x.shape == (B, N, C), x.shape
    nc = _get_nc()
    in_maps = make_in_maps(x, Wq, Wkv, sr_w, sr_b, ln_g, ln_b, Wp, bp)
    res = run_bass_kernel_spmd(nc, in_maps, core_ids=list(range(8)))
    out = np.empty((B, N, C), np.float32)
    for i in range(B):
        out[i, _ORIG_ROWS, :] = np.asarray(res.results[i]["out"], np.float32)
    return out
